# revision 18
# baseline (speedup 1.0000x reference)
"""Causal multi-head attention (B=2, S=2048, D=1024, H=16, Dh=64) on 8 trn2 cores.

Sharding: (batch, head-group) tensor parallel. Core c handles batch c//4 and
heads [4*(c%4), 4*(c%4)+4). Each core computes its 4 heads end-to-end
(QKV projections, causal softmax attention, W_O projection) and returns a
partial [S, D] output; the host sums the 4 partials per batch.

Per-core dataflow:
  - Q^T, K^T produced in [Dh, S] layout so scores come out transposed
    (S^T[k, q]) and the softmax'd P~ needs no transpose for the P@V matmul.
  - Softmax denominator via a ones-column appended to V (M=65 matmuls):
    row 64 of the attention PSUM is the denominator.
  - Causal mask = additive -1e30 upper-triangular tile applied to the
    diagonal PSUM scores blocks before exp.
  - W_O projection consumes the normalized attn^T directly as the stationary
    operand, accumulating both head-pairs in PSUM.

Engine balance (measured): ACT exp is the attention-phase bottleneck
(1 elem/lane/cycle @1.2GHz + ~300ns/instruction), and an ACT-gated PE
re-throttles to 1.2GHz (HAM). So:
  - QKV projections (PE-dense) are interleaved with attention per s-quarter:
    attention for q-chunk qc only needs Q/K/V quarters <= qc.
  - One exp instruction covers both head-parities of a kt block (scores land
    in one two-bank [128,1024] PSUM tile).
  - attn matmul for kt runs one kt behind its scores/exp; W_O of chunk qc
    runs during chunk qc+1.
  - PSUM->SBUF copies go to DVE, keeping ACT exp-only.
  - bf16 operands for QKV/scores/attn matmuls (enables fast weight load so
    LDWEIGHTS hides; softmax scale /8 keeps the bf16 scores error ~2e-3);
    fp32 PSUM accumulation everywhere; W_O in float32r.
"""

import numpy as np

try:
    import concourse  # noqa: F401
except ImportError:  # pragma: no cover - harness containers stage it here
    import sys

    sys.path.insert(0, "/opt/trn_rl_repo")

B, S, D, H, DH = 2, 2048, 1024, 16, 64
NCORES = 8
HPC = 4  # heads per core
NPAIR = 2  # head pairs per core
SC = 512  # q-chunk width (scores matmul N)
NQC = S // SC  # 4 q-chunks
NST = S // 128  # 16 s/k/q tiles of 128
NDC = D // 128  # 8 contraction chunks of 128
VO_W = 65  # V columns + ones column
VO_QSTRIDE = 4 * VO_W  # per-head stride inside one quarter's V|ones tile

_cache = {}


def _build_program():
    from contextlib import ExitStack

    import concourse.mybir as mybir
    import concourse.tile as tile
    from concourse import bacc

    f32 = mybir.dt.float32
    f32r = mybir.dt.float32r
    bf16 = mybir.dt.bfloat16
    AF = mybir.ActivationFunctionType

    nc = bacc.Bacc(
        "TRN2", debug=False, target_bir_lowering=False, num_devices=NCORES
    )

    xT = nc.dram_tensor("xT", [D, S], bf16, kind="ExternalInput").ap()
    wqk = nc.dram_tensor(
        "wqk", [128, 4 * NDC * 128], bf16, kind="ExternalInput"
    ).ap()
    wv = nc.dram_tensor("wv", [128, NDC * 256], bf16, kind="ExternalInput").ap()
    wo = nc.dram_tensor("wo", [128, NPAIR * D], f32r, kind="ExternalInput").ap()
    tri = nc.dram_tensor("tri", [128, 128], bf16, kind="ExternalInput").ap()
    out = nc.dram_tensor("out", [S, D], f32, kind="ExternalOutput").ap()

    with tile.TileContext(nc) as tc, ExitStack() as ctx:
        persist = ctx.enter_context(tc.tile_pool(name="persist", bufs=1))
        pt_pool = ctx.enter_context(tc.tile_pool(name="pt", bufs=6))
        den_pool = ctx.enter_context(tc.tile_pool(name="den", bufs=2))
        out_pool = ctx.enter_context(tc.tile_pool(name="outsb", bufs=2))
        ps_pool = ctx.enter_context(tc.tile_pool(name="ps", bufs=2, space="PSUM"))
        pa_pool = ctx.enter_context(tc.tile_pool(name="pa", bufs=2, space="PSUM"))

        # ---- persistent SBUF tensors (per s-quarter where it matters) ----
        x_sb = {
            (dc, q): persist.tile(
                [128, SC], bf16, tag=f"x{dc}_{q}", name=f"x{dc}_{q}"
            )
            for dc in range(NDC)
            for q in range(NQC)
        }
        wqk_sb = persist.tile([128, 4 * NDC * 128], bf16, tag="wqk", name="wqk_sb")
        wv_sb = persist.tile([128, NDC * 256], bf16, tag="wv", name="wv_sb")
        wo_sb = persist.tile([128, NPAIR * D], f32r, tag="wo", name="wo_sb")
        trib_sb = persist.tile([128, 128], bf16, tag="trib", name="trib_sb")
        ones_sb = persist.tile([128, 1], f32, tag="ones", name="ones_sb")
        qt_sb = {
            (p, q): persist.tile([128, SC], bf16, tag=f"qt{p}_{q}", name=f"qt{p}_{q}")
            for p in range(NPAIR)
            for q in range(NQC)
        }
        kt_sb = {
            (p, q): persist.tile([128, SC], bf16, tag=f"kt{p}_{q}", name=f"kt{p}_{q}")
            for p in range(NPAIR)
            for q in range(NQC)
        }
        vo_sb = {
            q: persist.tile(
                [128, HPC * VO_QSTRIDE], bf16, tag=f"vo{q}", name=f"vo{q}"
            )
            for q in range(NQC)
        }
        at_sb = {
            (p, qc): persist.tile(
                [128, SC], f32r, tag=f"at{p}_{qc}", name=f"at{p}_{qc}"
            )
            for p in range(NPAIR)
            for qc in range(NQC)
        }

        # ---- loads: x quarter 0 first so compute starts immediately ----
        for dc in range(NDC):
            nc.sync.dma_start(
                x_sb[(dc, 0)][:], xT[dc * 128 : (dc + 1) * 128, 0:SC]
            )
        BW = NDC * 128
        nc.sync.dma_start(wqk_sb[:, 0:BW], wqk[:, 0:BW])
        for blk in range(1, 4):
            nc.sync.dma_start(
                wqk_sb[:, blk * BW : (blk + 1) * BW],
                wqk[:, blk * BW : (blk + 1) * BW],
            )
        nc.sync.dma_start(wv_sb[:], wv[:])
        nc.sync.dma_start(trib_sb[:], tri[:])
        for q in range(1, NQC):
            for dc in range(NDC):
                nc.sync.dma_start(
                    x_sb[(dc, q)][:],
                    xT[dc * 128 : (dc + 1) * 128, q * SC : (q + 1) * SC],
                )
        nc.sync.dma_start(wo_sb[:], wo[:])
        nc.vector.memset(ones_sb[:], 1.0)
        for q in range(NQC):
            ones_cols = vo_sb[q].rearrange(
                "p (h s w) -> p h s w", h=HPC, w=VO_W
            )[:, :, :, 64]
            nc.vector.tensor_copy(
                ones_cols, ones_sb[:].to_broadcast((128, HPC, 4))
            )

        def emit_qkv(q):
            for p in range(NPAIR):
                for qk, dst in ((0, qt_sb[(p, q)]), (1, kt_sb[(p, q)])):
                    ps = ps_pool.tile(
                        [128, SC], f32, tag="ps", name=f"psqk{p}{qk}{q}"
                    )
                    for dc in range(NDC):
                        col = ((qk * NPAIR + p) * NDC + dc) * 128
                        nc.tensor.matmul(
                            ps[:, 0:SC],
                            lhsT=wqk_sb[:, col : col + 128],
                            rhs=x_sb[(dc, q)][:],
                            start=(dc == 0),
                            stop=(dc == NDC - 1),
                        )
                    nc.scalar.copy(dst[:], ps[:, 0:SC])
            for st4 in range(4):
                ps = ps_pool.tile([128, 256], f32, tag="ps", name=f"psv{q}{st4}")
                for dc in range(NDC):
                    nc.tensor.matmul(
                        ps[:],
                        lhsT=x_sb[(dc, q)][:, st4 * 128 : (st4 + 1) * 128],
                        rhs=wv_sb[:, dc * 256 : (dc + 1) * 256],
                        start=(dc == 0),
                        stop=(dc == NDC - 1),
                    )
                vo_cols = vo_sb[q].rearrange(
                    "p (h s w) -> p h s w", h=HPC, w=VO_W
                )[:, :, st4, 0:64]
                nc.scalar.copy(
                    vo_cols, ps[:].rearrange("p (h e) -> p h e", e=64)
                )

        def wo_thunks(qc):
            thunks = []
            for qt in range(4):
                thunks.append(lambda qt=qt, qc=qc: _emit_wo_qt(qc, qt))
            return thunks

        def _emit_wo_qt(qc, qt):
            if True:
                po = [
                    ps_pool.tile([128, SC], f32, tag="ps", name=f"po{qc}{qt}{dc}")
                    for dc in range(2)
                ]
                for p in range(NPAIR):
                    for dc in range(2):
                        nc.tensor.matmul(
                            po[dc][:],
                            lhsT=at_sb[(p, qc)][:, qt * 128 : (qt + 1) * 128],
                            rhs=wo_sb[:, p * D + dc * SC : p * D + (dc + 1) * SC],
                            start=(p == 0),
                            stop=(p == NPAIR - 1),
                        )
                outt = out_pool.tile([128, D], f32, tag="outsb", name=f"o{qc}{qt}")
                nc.scalar.copy(outt[:, 0:SC], po[0][:])
                nc.vector.tensor_copy(outt[:, SC:D], po[1][:])
                row = (qc * 4 + qt) * 128
                nc.sync.dma_start(out[row : row + 128, :], outt[:])

        def emit_attention(qc, wo_pending=()):
            wo_pending = list(wo_pending)
            nkt = 4 * (qc + 1)
            pa_qc = {
                p: pa_pool.tile([VO_W, 2 * SC], f32, tag="pa", name=f"pa{qc}{p}")
                for p in range(NPAIR)
            }

            def flush(p, kt, ptile):
                j0 = max(0, kt * 128 - qc * SC)
                kq, kst = kt // 4, kt % 4
                for par in range(2):
                    hh = 2 * p + par
                    vbase = hh * VO_QSTRIDE + kst * VO_W
                    nc.tensor.matmul(
                        pa_qc[p][:, par * SC + j0 : (par + 1) * SC],
                        lhsT=vo_sb[kq][:, vbase : vbase + VO_W],
                        rhs=ptile[:, par * SC + j0 : (par + 1) * SC],
                        start=(kt == 0),
                        stop=(kt == nkt - 1),
                    )

            # both pairs advance kt together: two independent
            # scores->exp->attn chains keep ACT continuously fed
            pending = []  # (p, kt, ptile) awaiting the P@V matmul
            for kt in range(nkt):
                j0 = max(0, kt * 128 - qc * SC)
                kq, kst = kt // 4, kt % 4
                for p in range(NPAIR):
                    ps_s = ps_pool.tile(
                        [128, 2 * SC], f32, tag="ps", name=f"pss{qc}{p}{kt}"
                    )
                    for par in range(2):
                        nc.tensor.matmul(
                            ps_s[:, par * SC + j0 : (par + 1) * SC],
                            lhsT=kt_sb[(p, kq)][
                                par * 64 : (par + 1) * 64,
                                kst * 128 : (kst + 1) * 128,
                            ],
                            rhs=qt_sb[(p, qc)][par * 64 : (par + 1) * 64, j0:SC],
                            start=True,
                            stop=True,
                        )
                    ptile = pt_pool.tile(
                        [128, 2 * SC], bf16, tag="pt", name=f"pt{qc}{p}{kt}"
                    )
                    nc.scalar.activation(
                        ptile.rearrange("p (b n) -> p b n", b=2)[:, :, j0:SC],
                        ps_s.rearrange("p (b n) -> p b n", b=2)[:, :, j0:SC],
                        AF.Exp,
                        scale=0.125,
                    )
                    if kt * 128 >= qc * SC:  # diagonal blocks: causal mask on
                        # the exp'd bf16 tile (multiplicative, on idle GpSimd)
                        nc.gpsimd.tensor_mul(
                            ptile.rearrange("p (b n) -> p b n", b=2)[
                                :, :, j0 : j0 + 128
                            ],
                            ptile.rearrange("p (b n) -> p b n", b=2)[
                                :, :, j0 : j0 + 128
                            ],
                            trib_sb[:].unsqueeze(1).to_broadcast((128, 2, 128)),
                        )
                    pending.append((p, kt, ptile))
                if wo_pending:
                    wo_pending.pop(0)()
                while len(pending) > 4:
                    flush(*pending.pop(0))
            for pend in pending:
                flush(*pend)
            for t in wo_pending:
                t()

            # normalize both pairs: one batched reciprocal (rows at 0/32/64/96)
            den = den_pool.tile([97, SC], f32, tag="den", name=f"den{qc}")
            nc.vector.memset(den[:], 1.0)
            for p in range(NPAIR):
                for par in range(2):
                    i = 32 * (2 * p + par)
                    nc.vector.tensor_copy(
                        den[i : i + 1, :],
                        pa_qc[p][64:65, par * SC : (par + 1) * SC],
                    )
            den_r = den_pool.tile([97, SC], f32, tag="denr", name=f"denr{qc}")
            nc.vector.reciprocal(den_r[:], den[:])
            for p in range(NPAIR):
                for par in range(2):
                    i = 32 * (2 * p + par)
                    # partition_broadcast ucode reads partition 0 of the source
                    # tile regardless of AP base - bounce through a base-0 tile
                    den_s = den_pool.tile(
                        [1, SC], f32, tag="dens", name=f"dens{qc}{p}{par}"
                    )
                    nc.vector.tensor_copy(den_s[:], den_r[i : i + 1, :])
                    denb = den_pool.tile(
                        [64, SC], f32, tag="denb", name=f"denb{qc}{p}{par}"
                    )
                    nc.gpsimd.partition_broadcast(denb[:], den_s[:])
                    nc.vector.tensor_mul(
                        at_sb[(p, qc)][par * 64 : (par + 1) * 64, :],
                        pa_qc[p][0:64, par * SC : (par + 1) * SC],
                        denb[:],
                    )

        for q in range(NQC):
            emit_qkv(q)
            emit_attention(q, wo_thunks(q - 1) if q >= 1 else ())
        for t in wo_thunks(NQC - 1):
            t()

    nc.compile()
    return nc


def _get_program():
    if "nc" not in _cache:
        _cache["nc"] = _build_program()
    return _cache["nc"]


def _prep_core_inputs(c, residual, W_Q, W_K, W_V, W_O, tri):
    import ml_dtypes

    b = c // 4
    heads = [4 * (c % 4) + i for i in range(HPC)]

    def chunked(w):  # [1024, M] -> [128, NDC*M] chunk-major
        m = w.shape[1]
        return np.ascontiguousarray(
            w.reshape(NDC, 128, m).transpose(1, 0, 2).reshape(128, NDC * m)
        )

    wqk_blocks = []
    for Wt in (W_Q, W_K):
        for p in range(NPAIR):
            h0, h1 = heads[2 * p], heads[2 * p + 1]
            wpair = np.concatenate([Wt[h0].T, Wt[h1].T], axis=1)  # [1024, 128]
            wqk_blocks.append(chunked(wpair))
    wqk_arr = np.ascontiguousarray(np.concatenate(wqk_blocks, axis=1))

    wv_arr = chunked(np.concatenate([W_V[h].T for h in heads], axis=1))
    wo_arr = np.ascontiguousarray(
        np.concatenate(
            [
                np.concatenate([W_O[heads[2 * p]], W_O[heads[2 * p + 1]]], axis=0)
                for p in range(NPAIR)
            ],
            axis=1,
        )
    )
    return {
        "xT": np.ascontiguousarray(residual[b].T).astype(ml_dtypes.bfloat16),
        "wqk": wqk_arr.astype(ml_dtypes.bfloat16),
        "wv": wv_arr.astype(ml_dtypes.bfloat16),
        "wo": wo_arr,
        "tri": tri,
    }


def make_in_maps(residual, W_Q, W_K, W_V, W_O):
    residual = np.asarray(residual, np.float32)
    W_Q, W_K, W_V, W_O = (np.asarray(w, np.float32) for w in (W_Q, W_K, W_V, W_O))
    import ml_dtypes

    # multiplicative causal mask for S^T[k, q] diagonal blocks: keep j >= p
    tri = np.triu(np.ones((128, 128), np.float32)).astype(ml_dtypes.bfloat16)
    return [
        _prep_core_inputs(c, residual, W_Q, W_K, W_V, W_O, tri)
        for c in range(NCORES)
    ]


def gather(results):
    out = np.zeros((B, S, D), np.float64)
    for c in range(NCORES):
        out[c // 4] += results[c]["out"].astype(np.float64)
    return out.astype(np.float32)


def kernel(residual, W_Q, W_K, W_V, W_O, **run_kwargs):
    from concourse.bass_utils import run_bass_kernel_spmd

    nc = _get_program()
    in_maps = make_in_maps(residual, W_Q, W_K, W_V, W_O)
    res = run_bass_kernel_spmd(nc, in_maps, core_ids=list(range(NCORES)), **run_kwargs)
    out = gather(res.results)
    if run_kwargs:
        _cache["last_results"] = res
    return out
